# revision 14
# baseline (speedup 1.0000x reference)
"""Trainium2 Bass kernel: MHSA with multi-head relative position embedding.

Sharding: data-parallel over batch — 16 batches / 8 cores = 2 batches per core,
each core computes all 8 heads for its 2 batches. No collectives needed.

Math per batch (N=784 tokens, C=512, 8 heads x 64 dim):
  qkv = x @ w_qkv                  (q-columns pre-scaled by 1/8 on host)
  scores_T[k,q] = k_h^T q_h + biasT[h,k,q]   (bias Toeplitz-gathered on host,
                                              static rel_idx, passed transposed)
  E = exp(scores_T)  (no max-subtraction: |scores| < ~2, exp is safe)
  O_T[d,q] = sum_k v_aug[k, d] E[k,q]  with v_aug = [v | 1] -> row 64 = sumexp
  attnout_T = O_T[0:64] / O_T[64]
  out = attnout^T stacked over heads @ w_out

Device layouts: everything transposed (features on partitions) so q^T/k^T are
natural slices of the qkv^T projection; v in token-major from the same x^T via
swapping matmul operands. attn@v is done as (v_aug^T @ E) to avoid transposing
the 784x784 attention matrix; softmax denominator comes free as the ones-column
row of the augmented v.
"""

import numpy as np
import ml_dtypes

B, HH, WW, C = 16, 28, 28, 512
N = HH * WW            # 784 tokens
HEADS, KD = 8, 64
NCORES, BPC = 8, 2     # 8 cores, 2 batches per core
NT, TP = 7, 112        # 784 = 7 tiles of 112 (k / token tiling)
CHUNKS = [(0, 512), (512, 272)]   # q-chunks (PSUM bank = 512 fp32)
CT = 4                 # contraction tiles of 128 over C=512
F_QK_TILES = 8         # 4 q-feature + 4 k-feature tiles of 128

_CACHE = {}


def _rel_index():
    # Faithful to reference._relative_position_index: token r -> (r%28, r//28)
    t = np.arange(N)
    c0, c1 = t % HH, t // HH
    return ((c0[:, None] - c0[None, :] + HH - 1)
            + (c1[:, None] - c1[None, :] + WW - 1) * (2 * HH - 1))  # [q, k]


def build_nc():
    if 'nc' in _CACHE:
        return _CACHE['nc']
    from contextlib import ExitStack
    import concourse.bacc as bacc
    import concourse.mybir as mybir
    import concourse.tile as tile
    from concourse.alu_op_type import AluOpType

    f32 = mybir.dt.float32
    bf16 = mybir.dt.bfloat16
    EXP = mybir.ActivationFunctionType.Exp

    nc = bacc.Bacc("TRN2", debug=False, enable_asserts=False)
    xT_d = nc.dram_tensor("xT", [BPC, C, N], bf16, kind="ExternalInput").ap()
    wqkv_d = nc.dram_tensor("wqkv", [C, 3 * C], bf16, kind="ExternalInput").ap()
    wout_d = nc.dram_tensor("wout", [C, C], bf16, kind="ExternalInput").ap()
    bias_d = nc.dram_tensor("biasT", [HEADS, N, N], bf16, kind="ExternalInput").ap()
    out_d = nc.dram_tensor("out", [BPC, N, C], f32, kind="ExternalOutput").ap()

    with tile.TileContext(nc) as tc, ExitStack() as ctx:
        persist = ctx.enter_context(tc.tile_pool(name="persist", bufs=1))
        xT_pool = ctx.enter_context(tc.tile_pool(name="xTp", bufs=8))
        bias_pool = ctx.enter_context(tc.tile_pool(name="biasp", bufs=21))
        eraw_pool = ctx.enter_context(tc.tile_pool(name="erp", bufs=8))
        e_pool = ctx.enter_context(tc.tile_pool(name="ep", bufs=8))
        r_pool = ctx.enter_context(tc.tile_pool(name="rp", bufs=4))
        osb_pool = ctx.enter_context(tc.tile_pool(name="osbp", bufs=2))
        sc_psum = ctx.enter_context(tc.tile_pool(name="scp", bufs=4, space="PSUM"))
        o_psum = ctx.enter_context(tc.tile_pool(name="opp", bufs=4, space="PSUM"))
        pj_psum = o_psum  # share banks: proj phases and attention never overlap much

        # ---- weights resident in SBUF ----
        wqkv_sb, wout_sb = [], []
        for ci in range(CT):
            w = persist.tile([128, 3 * C], bf16, tag=f"wqkv{ci}")
            nc.sync.dma_start(w, wqkv_d[ci * 128:(ci + 1) * 128, :])
            wqkv_sb.append(w)
        for ci in range(CT):
            w = persist.tile([128, C], bf16, tag=f"wout{ci}")
            nc.sync.dma_start(w, wout_d[ci * 128:(ci + 1) * 128, :])
            wout_sb.append(w)

        qkT, vsb, attnT = {}, {}, {}
        for b in range(BPC):
            for fi in range(CT):
                attnT[b, fi] = persist.tile(
                    [128, N], bf16, tag=f"attnT{b}_{fi}", name=f"attnT{b}_{fi}")

        # ---- qkv projection (per batch) ----
        for b in range(BPC):
            with nc.named_scope(f"qkv_b{b}"):
                xT_sb = []
                for ci in range(CT):
                    xt = xT_pool.tile([128, N], bf16, tag="xT")
                    nc.sync.dma_start(xt, xT_d[b, ci * 128:(ci + 1) * 128, :])
                    xT_sb.append(xt)
                # q^T / k^T: features on partitions  (out = w_slice.T @ xT)
                for ft in range(F_QK_TILES):
                    dst = persist.tile([128, N], bf16, tag=f"qkT{b}_{ft}")
                    qkT[b, ft] = dst
                    for (c0w, cw) in CHUNKS:
                        ps = pj_psum.tile([128, cw], f32, tag="op")
                        for ci in range(CT):
                            nc.tensor.matmul(
                                ps, wqkv_sb[ci][:, ft * 128:(ft + 1) * 128],
                                xT_sb[ci][:, c0w:c0w + cw],
                                start=(ci == 0), stop=(ci == CT - 1))
                        nc.vector.tensor_copy(dst[:, c0w:c0w + cw], ps)
                # v: token-major [112, 8, 65] with ones column at d=64
                for t in range(NT):
                    vt = persist.tile([TP, HEADS, KD + 1], bf16, tag=f"v{b}_{t}")
                    vsb[b, t] = vt
                    ps = pj_psum.tile([TP, C], f32, tag="op")
                    for ci in range(CT):
                        nc.tensor.matmul(
                            ps, xT_sb[ci][:, t * TP:(t + 1) * TP],
                            wqkv_sb[ci][:, 2 * C:3 * C],
                            start=(ci == 0), stop=(ci == CT - 1))
                    nc.vector.tensor_copy(
                        vt[:, :, 0:KD], ps.rearrange("p (h d) -> p h d", h=HEADS))
                    nc.vector.memset(vt[:, :, KD:KD + 1], 1.0)

        # ---- attention (head-outer so exp(bias) loads once per head) ----
        # scores+bias handled as exp(s+b) = exp(s) * exp(b): ACT exp reads the
        # scores PSUM directly, then one fast bf16 SBUF multiply by the
        # host-precomputed exp(bias).
        for hp in range(HEADS // 2):
            h0, h1 = 2 * hp, 2 * hp + 1
            with nc.named_scope(f"headpair{hp}"):
                bias_sb = {}
                for h in (h0, h1):
                    for kt in range(NT):
                        bt = bias_pool.tile([TP, N], bf16, tag="bias",
                                            name=f"bias{h}_{kt}")
                        nc.sync.dma_start(bt, bias_d[h, kt * TP:(kt + 1) * TP, :])
                        bias_sb[h, kt] = bt
                for (c0w, cw) in CHUNKS:
                    # 4 streams: (head even/odd) x (batch) — adjacent even/odd
                    # scores matmuls hit disjoint PE row-groups (base partition
                    # 0 vs 64) and run concurrently; v-matmul trails 1 kt.
                    ops, esbs = {}, {}
                    for h in (h0, h1):
                        for b in range(BPC):
                            ops[h, b] = o_psum.tile(
                                [KD + 1, cw], f32, tag="op",
                                name=f"op{h}_{c0w}_{b}")
                    for kt in range(NT):
                        for b in range(BPC):
                            for h in (h0, h1):
                                r0 = (h % 2) * 64
                                kT_tile = qkT[b, 4 + h // 2]
                                qT_tile = qkT[b, h // 2]
                                scp = sc_psum.tile([TP, cw], f32, tag="sc",
                                                   name=f"sc{h}_{c0w}_{b}_{kt}")
                                nc.tensor.matmul(
                                    scp,
                                    kT_tile[r0:r0 + 64, kt * TP:(kt + 1) * TP],
                                    qT_tile[r0:r0 + 64, c0w:c0w + cw],
                                    start=True, stop=True)
                                eraw = eraw_pool.tile(
                                    [TP, cw], bf16, tag="eraw",
                                    name=f"er{h}_{c0w}_{b}_{kt}")
                                nc.scalar.activation(eraw, scp, EXP)
                                esb = e_pool.tile([TP, cw], bf16, tag="e",
                                                  name=f"e{h}_{c0w}_{b}_{kt}")
                                nc.vector.tensor_tensor(
                                    esb, eraw, bias_sb[h, kt][:, c0w:c0w + cw],
                                    AluOpType.mult)
                                esbs[h, b, kt] = esb
                        if kt >= 1:
                            for b in range(BPC):
                                for h in (h0, h1):
                                    nc.tensor.matmul(
                                        ops[h, b], vsb[b, kt - 1][:, h:h + 1, :],
                                        esbs[h, b, kt - 1],
                                        start=(kt == 1), stop=False)
                    for b in range(BPC):
                        for h in (h0, h1):
                            nc.tensor.matmul(
                                ops[h, b], vsb[b, NT - 1][:, h:h + 1, :],
                                esbs[h, b, NT - 1], start=False, stop=True)
                    for b in range(BPC):
                        for h in (h0, h1):
                            r0 = (h % 2) * 64
                            # normalize: rows 0..63 * (1 / row 64). Copy O out
                            # of PSUM first so the accumulation bank frees for
                            # the next chunk immediately instead of after the
                            # whole recip->broadcast->mult chain.
                            # (approx-recip must not read PSUM directly: its
                            # bitwise seed sees raw PSUM bits on HW -> garbage)
                            onorm = r_pool.tile([KD, 512], f32, tag="onorm")
                            nc.vector.tensor_copy(
                                onorm[:, 0:cw], ops[h, b][0:KD, 0:cw])
                            srow = r_pool.tile([1, 512], f32, tag="srow")
                            nc.vector.tensor_copy(
                                srow[:, 0:cw], ops[h, b][KD:KD + 1, 0:cw])
                            rrow = r_pool.tile([1, 512], f32, tag="rrow")
                            nc.vector.reciprocal_approx_fast(
                                rrow[:, 0:cw], srow[:, 0:cw])
                            rb = r_pool.tile([64, 512], f32, tag="rb")
                            nc.gpsimd.partition_broadcast(
                                rb[:, 0:cw], rrow[:, 0:cw])
                            nc.vector.tensor_tensor(
                                attnT[b, h // 2][r0:r0 + 64, c0w:c0w + cw],
                                onorm[:, 0:cw], rb[:, 0:cw],
                                AluOpType.mult)

        # ---- output projection ----
        for b in range(BPC):
            with nc.named_scope(f"proj_b{b}"):
                for t in range(NT):
                    ps = pj_psum.tile([TP, C], f32, tag="op")
                    for fi in range(CT):
                        nc.tensor.matmul(
                            ps, attnT[b, fi][:, t * TP:(t + 1) * TP], wout_sb[fi],
                            start=(fi == 0), stop=(fi == CT - 1))
                    osb = osb_pool.tile([TP, C], f32, tag="osb")
                    nc.vector.tensor_copy(osb, ps)
                    nc.sync.dma_start(out_d[b, t * TP:(t + 1) * TP, :], osb)

    nc.compile()
    _CACHE['nc'] = nc
    return nc


def host_prep(x, w_qkv, pos_table, w_out):
    x = np.asarray(x, np.float32).reshape(B, N, C)
    wq = np.array(np.asarray(w_qkv, np.float32), copy=True)
    wq[:, :C] *= np.float32(1.0 / np.sqrt(KD))
    wq_bf = wq.astype(ml_dtypes.bfloat16)
    idx = _rel_index()
    biasT = np.ascontiguousarray(np.exp(
        np.asarray(pos_table, np.float32)[:, idx].transpose(0, 2, 1)
    )).astype(ml_dtypes.bfloat16)
    wout = np.ascontiguousarray(np.asarray(w_out, np.float32)).astype(
        ml_dtypes.bfloat16)
    in_maps = []
    for c in range(NCORES):
        xT = np.ascontiguousarray(
            x[c * BPC:(c + 1) * BPC].transpose(0, 2, 1)).astype(
                ml_dtypes.bfloat16)  # [2, 512, 784]
        in_maps.append({"xT": xT, "wqkv": wq_bf, "wout": wout, "biasT": biasT})
    return in_maps


def run(in_maps, trace=False, trace_cores=None):
    import concourse.bass_utils as bass_utils
    nc = build_nc()
    return bass_utils.run_bass_kernel_spmd(
        nc, in_maps, core_ids=list(range(NCORES)),
        trace=trace, trace_cores=trace_cores)


def kernel(x, w_qkv, pos_table, w_out):
    in_maps = host_prep(x, w_qkv, pos_table, w_out)
    res = run(in_maps)
    out = np.stack([r["out"] for r in res.results])    # [8, 2, 784, 512]
    return np.ascontiguousarray(out.reshape(B, HH, WW, C)).astype(np.float32)


# revision 16
# speedup vs baseline: 1.0700x; 1.0700x over previous
"""Trainium2 Bass kernel: MHSA with multi-head relative position embedding.

Sharding: data-parallel over batch — 16 batches / 8 cores = 2 batches per core,
each core computes all 8 heads for its 2 batches. No collectives needed.

Math per batch (N=784 tokens, C=512, 8 heads x 64 dim):
  qkv = x @ w_qkv                  (q-columns pre-scaled by 1/8 on host)
  scores_T[k,q] = k_h^T q_h + biasT[h,k,q]   (bias Toeplitz-gathered on host,
                                              static rel_idx, passed transposed)
  E = exp(scores_T)  (no max-subtraction: |scores| < ~2, exp is safe)
  O_T[d,q] = sum_k v_aug[k, d] E[k,q]  with v_aug = [v | 1] -> row 64 = sumexp
  attnout_T = O_T[0:64] / O_T[64]
  out = attnout^T stacked over heads @ w_out

Device layouts: everything transposed (features on partitions) so q^T/k^T are
natural slices of the qkv^T projection; v in token-major from the same x^T via
swapping matmul operands. attn@v is done as (v_aug^T @ E) to avoid transposing
the 784x784 attention matrix; softmax denominator comes free as the ones-column
row of the augmented v.
"""

import numpy as np
import ml_dtypes

B, HH, WW, C = 16, 28, 28, 512
N = HH * WW            # 784 tokens
HEADS, KD = 8, 64
NCORES, BPC = 8, 2     # 8 cores, 2 batches per core
NT, TP = 7, 112        # 784 = 7 tiles of 112 (k / token tiling)
CHUNKS = [(0, 512), (512, 272)]   # q-chunks (PSUM bank = 512 fp32)
CT = 4                 # contraction tiles of 128 over C=512
F_QK_TILES = 8         # 4 q-feature + 4 k-feature tiles of 128

_CACHE = {}


def _rel_index():
    # Faithful to reference._relative_position_index: token r -> (r%28, r//28)
    t = np.arange(N)
    c0, c1 = t % HH, t // HH
    return ((c0[:, None] - c0[None, :] + HH - 1)
            + (c1[:, None] - c1[None, :] + WW - 1) * (2 * HH - 1))  # [q, k]


def build_nc():
    if 'nc' in _CACHE:
        return _CACHE['nc']
    from contextlib import ExitStack
    import concourse.bacc as bacc
    import concourse.mybir as mybir
    import concourse.tile as tile
    from concourse.alu_op_type import AluOpType

    f32 = mybir.dt.float32
    bf16 = mybir.dt.bfloat16
    EXP = mybir.ActivationFunctionType.Exp

    nc = bacc.Bacc("TRN2", debug=False, enable_asserts=False)
    xT_d = nc.dram_tensor("xT", [BPC, C, N], bf16, kind="ExternalInput").ap()
    wqkv_d = nc.dram_tensor("wqkv", [C, 3 * C], bf16, kind="ExternalInput").ap()
    wout_d = nc.dram_tensor("wout", [C, C], bf16, kind="ExternalInput").ap()
    bias_d = nc.dram_tensor("biasT", [HEADS, N, N], bf16, kind="ExternalInput").ap()
    out_d = nc.dram_tensor("out", [BPC, N, C], f32, kind="ExternalOutput").ap()

    with tile.TileContext(nc) as tc, ExitStack() as ctx:
        persist = ctx.enter_context(tc.tile_pool(name="persist", bufs=1))
        xT_pool = ctx.enter_context(tc.tile_pool(name="xTp", bufs=8))
        bias_pool = ctx.enter_context(tc.tile_pool(name="biasp", bufs=21))
        eraw_pool = ctx.enter_context(tc.tile_pool(name="erp", bufs=8))
        e_pool = ctx.enter_context(tc.tile_pool(name="ep", bufs=8))
        r_pool = ctx.enter_context(tc.tile_pool(name="rp", bufs=3))
        osb_pool = ctx.enter_context(tc.tile_pool(name="osbp", bufs=2))
        sc_psum = ctx.enter_context(tc.tile_pool(name="scp", bufs=4, space="PSUM"))
        o_psum = ctx.enter_context(tc.tile_pool(name="opp", bufs=4, space="PSUM"))
        pj_psum = o_psum  # share banks: proj phases and attention never overlap much

        # ---- weights resident in SBUF ----
        wqkv_sb, wout_sb = [], []
        for ci in range(CT):
            w = persist.tile([128, 3 * C], bf16, tag=f"wqkv{ci}")
            nc.sync.dma_start(w, wqkv_d[ci * 128:(ci + 1) * 128, :])
            wqkv_sb.append(w)
        for ci in range(CT):
            w = persist.tile([128, C], bf16, tag=f"wout{ci}")
            nc.sync.dma_start(w, wout_d[ci * 128:(ci + 1) * 128, :])
            wout_sb.append(w)

        qkT, vsb, attnT = {}, {}, {}
        for b in range(BPC):
            for fi in range(CT):
                attnT[b, fi] = persist.tile(
                    [128, N], bf16, tag=f"attnT{b}_{fi}", name=f"attnT{b}_{fi}")

        # ---- qkv projection, split so head-pair 0 can start early ----
        def emit_xt(b):
            tiles = []
            for ci in range(CT):
                xt = xT_pool.tile([128, N], bf16, tag="xT", name=f"xT{b}_{ci}")
                nc.sync.dma_start(xt, xT_d[b, ci * 128:(ci + 1) * 128, :])
                tiles.append(xt)
            return tiles

        def emit_qk_tile(b, ft, xT_sb):
            dst = persist.tile([128, N], bf16, tag=f"qkT{b}_{ft}",
                               name=f"qkT{b}_{ft}")
            qkT[b, ft] = dst
            for (c0w, cw) in CHUNKS:
                ps = pj_psum.tile([128, cw], f32, tag="op", name=f"pj{b}_{ft}_{c0w}")
                for ci in range(CT):
                    nc.tensor.matmul(
                        ps, wqkv_sb[ci][:, ft * 128:(ft + 1) * 128],
                        xT_sb[ci][:, c0w:c0w + cw],
                        start=(ci == 0), stop=(ci == CT - 1))
                nc.vector.tensor_copy(dst[:, c0w:c0w + cw], ps)

        def emit_v(b, xT_sb):
            for t in range(NT):
                vt = persist.tile([TP, HEADS, KD + 1], bf16, tag=f"v{b}_{t}",
                                  name=f"v{b}_{t}")
                vsb[b, t] = vt
                ps = pj_psum.tile([TP, C], f32, tag="op", name=f"pv{b}_{t}")
                for ci in range(CT):
                    nc.tensor.matmul(
                        ps, xT_sb[ci][:, t * TP:(t + 1) * TP],
                        wqkv_sb[ci][:, 2 * C:3 * C],
                        start=(ci == 0), stop=(ci == CT - 1))
                nc.vector.tensor_copy(
                    vt[:, :, 0:KD], ps.rearrange("p (h d) -> p h d", h=HEADS))
                nc.vector.memset(vt[:, :, KD:KD + 1], 1.0)

        # ---- attention for one head pair ----
        def attention_pair(hp):
            h0, h1 = 2 * hp, 2 * hp + 1
            with nc.named_scope(f"headpair{hp}"):
                bias_sb = {}
                for h in (h0, h1):
                    for kt in range(NT):
                        bt = bias_pool.tile([TP, N], bf16, tag="bias",
                                            name=f"bias{h}_{kt}")
                        nc.sync.dma_start(bt, bias_d[h, kt * TP:(kt + 1) * TP, :])
                        bias_sb[h, kt] = bt
                for (c0w, cw) in CHUNKS:
                    # 4 streams: (head even/odd) x (batch) — adjacent even/odd
                    # scores matmuls hit disjoint PE row-groups (base partition
                    # 0 vs 64) and run concurrently; v-matmul trails 1 kt.
                    ops, esbs = {}, {}
                    for h in (h0, h1):
                        for b in range(BPC):
                            ops[h, b] = o_psum.tile(
                                [KD + 1, cw], f32, tag="op",
                                name=f"op{h}_{c0w}_{b}")
                    for kt in range(NT):
                        for b in range(BPC):
                            for h in (h0, h1):
                                r0 = (h % 2) * 64
                                kT_tile = qkT[b, 4 + h // 2]
                                qT_tile = qkT[b, h // 2]
                                scp = sc_psum.tile([TP, cw], f32, tag="sc",
                                                   name=f"sc{h}_{c0w}_{b}_{kt}")
                                nc.tensor.matmul(
                                    scp,
                                    kT_tile[r0:r0 + 64, kt * TP:(kt + 1) * TP],
                                    qT_tile[r0:r0 + 64, c0w:c0w + cw],
                                    start=True, stop=True)
                                eraw = eraw_pool.tile(
                                    [TP, cw], bf16, tag="eraw",
                                    name=f"er{h}_{c0w}_{b}_{kt}")
                                nc.scalar.activation(eraw, scp, EXP)
                                esb = e_pool.tile([TP, cw], bf16, tag="e",
                                                  name=f"e{h}_{c0w}_{b}_{kt}")
                                nc.vector.tensor_tensor(
                                    esb, eraw, bias_sb[h, kt][:, c0w:c0w + cw],
                                    AluOpType.mult)
                                esbs[h, b, kt] = esb
                        if kt >= 1:
                            for b in range(BPC):
                                for h in (h0, h1):
                                    nc.tensor.matmul(
                                        ops[h, b], vsb[b, kt - 1][:, h:h + 1, :],
                                        esbs[h, b, kt - 1],
                                        start=(kt == 1), stop=False)
                    for b in range(BPC):
                        for h in (h0, h1):
                            nc.tensor.matmul(
                                ops[h, b], vsb[b, NT - 1][:, h:h + 1, :],
                                esbs[h, b, NT - 1], start=False, stop=True)
                    for b in range(BPC):
                        for h in (h0, h1):
                            r0 = (h % 2) * 64
                            # normalize: rows 0..63 * (1 / row 64)
                            # (approx-recip must not read PSUM directly: its
                            # bitwise seed sees raw PSUM bits on HW -> garbage)
                            srow = r_pool.tile([1, 512], f32, tag="srow")
                            nc.vector.tensor_copy(
                                srow[:, 0:cw], ops[h, b][KD:KD + 1, 0:cw])
                            rrow = r_pool.tile([1, 512], f32, tag="rrow")
                            nc.vector.reciprocal_approx_fast(
                                rrow[:, 0:cw], srow[:, 0:cw])
                            rb = r_pool.tile([64, 512], f32, tag="rb")
                            nc.gpsimd.partition_broadcast(
                                rb[:, 0:cw], rrow[:, 0:cw])
                            nc.vector.tensor_tensor(
                                attnT[b, h // 2][r0:r0 + 64, c0w:c0w + cw],
                                ops[h, b][0:KD, 0:cw], rb[:, 0:cw],
                                AluOpType.mult)

        # phase 1: minimal inputs for head-pair 0 (q-tile 0, k-tile 4, v)
        xts = {}
        for b in range(BPC):
            with nc.named_scope(f"qkv_early_b{b}"):
                xts[b] = emit_xt(b)
                emit_qk_tile(b, 0, xts[b])
                emit_qk_tile(b, 4, xts[b])
        for b in range(BPC):
            with nc.named_scope(f"v_b{b}"):
                emit_v(b, xts[b])
        # phase 2: pair-0 attention starts while the rest of qkv is emitted
        attention_pair(0)
        # phase 3: remaining q/k feature tiles (ACT is busy with pair 0 here)
        for b in range(BPC):
            with nc.named_scope(f"qkv_rest_b{b}"):
                for ft in (1, 5, 2, 6, 3, 7):
                    emit_qk_tile(b, ft, xts[b])
        # phase 4: remaining head pairs
        for hp in range(1, HEADS // 2):
            attention_pair(hp)

        # ---- output projection ----
        for b in range(BPC):
            with nc.named_scope(f"proj_b{b}"):
                for t in range(NT):
                    ps = pj_psum.tile([TP, C], f32, tag="op")
                    for fi in range(CT):
                        nc.tensor.matmul(
                            ps, attnT[b, fi][:, t * TP:(t + 1) * TP], wout_sb[fi],
                            start=(fi == 0), stop=(fi == CT - 1))
                    osb = osb_pool.tile([TP, C], f32, tag="osb")
                    nc.vector.tensor_copy(osb, ps)
                    nc.sync.dma_start(out_d[b, t * TP:(t + 1) * TP, :], osb)

    nc.compile()
    _CACHE['nc'] = nc
    return nc


def host_prep(x, w_qkv, pos_table, w_out):
    x = np.asarray(x, np.float32).reshape(B, N, C)
    wq = np.array(np.asarray(w_qkv, np.float32), copy=True)
    wq[:, :C] *= np.float32(1.0 / np.sqrt(KD))
    wq_bf = wq.astype(ml_dtypes.bfloat16)
    idx = _rel_index()
    biasT = np.ascontiguousarray(np.exp(
        np.asarray(pos_table, np.float32)[:, idx].transpose(0, 2, 1)
    )).astype(ml_dtypes.bfloat16)
    wout = np.ascontiguousarray(np.asarray(w_out, np.float32)).astype(
        ml_dtypes.bfloat16)
    in_maps = []
    for c in range(NCORES):
        xT = np.ascontiguousarray(
            x[c * BPC:(c + 1) * BPC].transpose(0, 2, 1)).astype(
                ml_dtypes.bfloat16)  # [2, 512, 784]
        in_maps.append({"xT": xT, "wqkv": wq_bf, "wout": wout, "biasT": biasT})
    return in_maps


def run(in_maps, trace=False, trace_cores=None):
    import concourse.bass_utils as bass_utils
    nc = build_nc()
    return bass_utils.run_bass_kernel_spmd(
        nc, in_maps, core_ids=list(range(NCORES)),
        trace=trace, trace_cores=trace_cores)


def kernel(x, w_qkv, pos_table, w_out):
    in_maps = host_prep(x, w_qkv, pos_table, w_out)
    res = run(in_maps)
    out = np.stack([r["out"] for r in res.results])    # [8, 2, 784, 512]
    return np.ascontiguousarray(out.reshape(B, HH, WW, C)).astype(np.float32)
